# revision 8
# baseline (speedup 1.0000x reference)
"""Trainium2 Bass kernel: causal attention (QKV projection + causal softmax + AV).

Problem: x[4, 4096, 768] fp32, per-head projections to d=64, full causal
attention per batch, output [4, 4096, 64] fp32.

Sharding: 8 cores = 4 batches x 2 parity groups. Core (b, j) computes the
output rows of batch b whose 128-row block index i satisfies i % 2 == j.
One uniform SPMD program: for j=0 cores the host shifts x down by one
128-row block (prepending zeros), which makes the causal structure of both
parities identical in device coordinates (device q-blocks are always the odd
blocks 1,3,...,31; k-slot g holds true block g-1 for j=0 and g for j=1; the
dead slot 0 of j=0 is zeroed post-exp with a per-core 0/1 scale).

Device pipeline per core (all matmuls bf16, fp32 PSUM accumulation):
  P1 (per 512-row seq chunk): one 3D-output DMA-transpose yields x^T for the
     chunk (SP HWDGE queue, issued first); two matmul passes with stationary
     [wq|wq] and [wv|wk] produce Q^T (own q-blocks, both partition halves),
     K^T (high half, HWDGE-duplicated even slots to the low half) and V^T
     (PE-transposed into V' = [V | 1]). Bias adds run on Pool.
  P2 (per 512-col q chunk): for consecutive k-slot pairs, two concurrent
     row-tiled matmuls K^T_g.T @ Q^T produce S^T; one merged exp on ACT
     (scale 1/8, AP [128, 2, w]); causal-diagonal 128x128 tri mask and the
     j=0 dead-slot kill on Pool; AV accumulates V'.T @ P^T into a [65, 512]
     PSUM tile whose row 64 is the softmax denominator. The inner loop is
     software-pipelined: S(p+1) is emitted before AV(p) so PE never waits
     on the exp. The unnormalized [65, 512] tiles go to DRAM; the host
     divides and transposes.
All non-transpose DMAs (weights, dups, output stores) ride the ACT HWDGE
queue, whose descriptor generation does not occupy the ACT execution unit.
"""

import numpy as np
import ml_dtypes
from contextlib import ExitStack

import concourse.bass as bass
import concourse.mybir as mybir
import concourse.tile as tile
from concourse import bacc
from concourse.bass_utils import run_bass_kernel_spmd

F32 = mybir.dt.float32
BF16 = mybir.dt.bfloat16

SEQ = 4096
DIN = 768
DOUT = 64
NCC = DIN // 128          # 6 contraction chunks
NSC = SEQ // 512          # 8 seq chunks (projection granularity)
NBLK = SEQ // 128         # 32 k-slots
NQC = 4                   # q chunks of 512 local columns (2048 own q rows)
SCALE = 1.0 / 8.0
EXPF = mybir.ActivationFunctionType.Exp

_CACHED_NC = None


def build_nc():
    nc = bacc.Bacc("TRN2", target_bir_lowering=False, debug=False)

    x = nc.dram_tensor("x", [SEQ, DIN], BF16, kind="ExternalInput")
    # host pre-interleaves weights to [in-chunk partition, (chunk, out_col)]
    wqq = nc.dram_tensor("wqq", [128, NCC * 128], BF16, kind="ExternalInput")
    wkv = nc.dram_tensor("wkv", [128, NCC * 128], BF16, kind="ExternalInput")
    bqq = nc.dram_tensor("bqq", [128, 1], F32, kind="ExternalInput")     # [bq;bq]
    pads = nc.dram_tensor("pads", [128, 1], F32, kind="ExternalInput")   # 1 / 0
    maska = nc.dram_tensor("maska", [128, 128], BF16, kind="ExternalInput")
    idnb = nc.dram_tensor("idnb", [64, 64], BF16, kind="ExternalInput")
    o = nc.dram_tensor("o", [NQC, 65, 512], F32, kind="ExternalOutput")

    with tile.TileContext(nc) as tc, ExitStack() as ctx:
        cpool = ctx.enter_context(tc.tile_pool(name="const", bufs=1))
        vtp = ctx.enter_context(tc.tile_pool(name="vt", bufs=2))
        ptp = ctx.enter_context(tc.tile_pool(name="pt", bufs=3))
        ocp = ctx.enter_context(tc.tile_pool(name="oc", bufs=2))
        psproj = ctx.enter_context(tc.tile_pool(name="psproj", bufs=2, space="PSUM"))
        psst = ctx.enter_context(tc.tile_pool(name="psst", bufs=2, space="PSUM"))
        psav = ctx.enter_context(tc.tile_pool(name="psav", bufs=2, space="PSUM"))

        wqq_sb = cpool.tile([128, NCC * 128], BF16)
        wkv_sb = cpool.tile([128, NCC * 128], BF16)
        bqq_sb = cpool.tile([128, 1], F32)
        pads_sb = cpool.tile([128, 1], F32)
        mask_sb = cpool.tile([128, 128], BF16)
        idn_sb = cpool.tile([64, 64], BF16)
        kt2 = cpool.tile([128, NBLK * 128], BF16)   # K^T, both partition halves
        xtf = cpool.tile([128, NSC * NCC * 512], BF16)  # x^T, whole sequence
        qt = cpool.tile([128, 16 * 128], BF16)      # Q^T own blocks, both halves
        vs = cpool.tile([128, NBLK * 65], BF16)     # V' = [V | 1] per k-slot

        # x^T transposes first, alone on the SP HWDGE queue
        for sc in range(NSC):
            nc.sync.dma_start_transpose(
                xtf[:, sc * NCC * 512:(sc + 1) * NCC * 512]
                .rearrange("p (cc s) -> p cc s", cc=NCC),
                x[sc * 512:(sc + 1) * 512, :],
            )
        # everything else on the ACT HWDGE queue
        nc.scalar.dma_start(wqq_sb[:], wqq[:, :])
        nc.scalar.dma_start(wkv_sb[:], wkv[:, :])
        nc.scalar.dma_start(bqq_sb[:], bqq[:, :])
        nc.scalar.dma_start(pads_sb[:], pads[:, :])
        nc.scalar.dma_start(mask_sb[:], maska[:, :])
        nc.scalar.dma_start(idn_sb[:], idnb[:, :])
        # ones column of V'
        nc.vector.memset(
            vs[:].rearrange("p (g e) -> p g e", g=NBLK)[:, :, 64:65], 1.0
        )

        def xts(sc, cc):
            base = sc * NCC * 512 + cc * 512
            return xtf[:, base:base + 512]

        def passA_chunk(sc):
            """Q^T for own (odd) q-blocks of this chunk, [wq|wq] stationary."""
            qp = psproj.tile([128, 256], F32, tag="proj")
            for cc in range(NCC):
                rhs = (
                    xts(sc, cc)
                    .rearrange("p (a b s) -> p a b s", a=2, b=2)[:, :, 1, :]
                )
                nc.tensor.matmul(
                    qp[:], wqq_sb[:, cc * 128:(cc + 1) * 128], rhs,
                    start=(cc == 0), stop=(cc == NCC - 1),
                )
            nc.vector.tensor_scalar_add(
                qt[:, sc * 256:(sc + 1) * 256], qp[:], bqq_sb[:]
            )

        def passB_chunk(sc):
            """K^T (rows 64-127) and V^T (rows 0-63), [wv|wk] stationary.

            The k/v biases are dropped: bk adds a per-query constant to the
            scores (softmax-invariant) and bv a constant to the output (the
            host adds it after normalizing). One bias-free copy evacuates
            the whole PSUM tile into kt2; V sits in the low half just long
            enough for the V' transposes, then the dup DMA overwrites the
            even k-slots of the low half with K for the S matmuls.
            """
            kp = psproj.tile([128, 512], F32, tag="proj")
            for cc in range(NCC):
                nc.tensor.matmul(
                    kp[:], wkv_sb[:, cc * 128:(cc + 1) * 128],
                    xts(sc, cc),
                    start=(cc == 0), stop=(cc == NCC - 1),
                )
            nc.vector.tensor_copy(
                kt2[:, sc * 512:(sc + 1) * 512], kp[:]
            )
            # V' blocks via PE transpose (DMA-transpose is only HW-exact for
            # the whole-row DRAM-sourced x case)
            vp = psproj.tile([128, 256], BF16, tag="proj")
            for t in range(4):
                nc.tensor.transpose(
                    vp[:, t * 64:(t + 1) * 64],
                    kt2[0:64, sc * 512 + t * 128: sc * 512 + (t + 1) * 128],
                    idn_sb[:],
                )
            nc.vector.tensor_copy(
                vs[:].rearrange("p (g e) -> p g e", g=NBLK)[
                    :, sc * 4:(sc + 1) * 4, 0:64
                ],
                vp[:].rearrange("p (g e) -> p g e", g=4),
            )
            hi = kt2[64:128, sc * 512:(sc + 1) * 512].rearrange(
                "p (a b s) -> p a b s", a=2, b=2)[:, :, 0, :]
            lo = kt2[0:64, sc * 512:(sc + 1) * 512].rearrange(
                "p (a b s) -> p a b s", a=2, b=2)[:, :, 0, :]
            nc.scalar.dma_start(lo, hi)

        def attn_seg(c):
            """Attention for local q cols [c*512, (c+1)*512), k-slot pairs 0..4c+3."""
            npairs = 4 * c + 4           # k-slots 0..8c+7 in consecutive pairs
            av = psav.tile([65, 512], F32, tag="av")

            def geom(p):
                # both slots of pair p share width w, regions left-aligned
                k = p - 4 * c
                return 512 - 128 * k if k > 0 else 512

            def s_pair(p):
                w = geom(p)
                g0, g1 = 2 * p, 2 * p + 1
                st = psst.tile([128, 1024], F32, tag="st")
                nc.tensor.matmul(
                    st[:, 0:w], kt2[0:64, g0 * 128:(g0 + 1) * 128],
                    qt[0:64, c * 512 + 512 - w: (c + 1) * 512],
                    start=True, stop=True, tile_position=(0, 0),
                )
                nc.tensor.matmul(
                    st[:, 512:512 + w], kt2[64:128, g1 * 128:(g1 + 1) * 128],
                    qt[64:128, c * 512 + 512 - w: (c + 1) * 512],
                    start=True, stop=True, tile_position=(64, 0),
                )
                return st

            def px_pair(p, st):
                w = geom(p)
                pt = ptp.tile([128, 1024], BF16)
                nc.scalar.activation(
                    pt[:].rearrange("p (i n) -> p i n", i=2)[:, :, 0:w],
                    st[:].rearrange("p (i n) -> p i n", i=2)[:, :, 0:w],
                    EXPF, bias=0.0, scale=SCALE,
                )
                if p == 0:
                    # kill the j=0 dead slot 0 (pads = 0 there, 1 for j=1)
                    nc.gpsimd.tensor_scalar_mul(
                        pt[:, 0:512], pt[:, 0:512], pads_sb[:]
                    )
                if p >= npairs - 4:
                    # odd member of the last four pairs is causal-diagonal:
                    # its first 128 written cols are the triangular block
                    nc.gpsimd.tensor_mul(
                        pt[:, 512:640], pt[:, 512:640], mask_sb[:]
                    )
                return pt

            def av_pair(p, pt):
                w = geom(p)
                off = 512 - w
                g0, g1 = 2 * p, 2 * p + 1
                nc.tensor.matmul(
                    av[:, off:512], vs[:, g0 * 65:(g0 + 1) * 65],
                    pt[:, 0:w],
                    start=(p == 0), stop=False,
                )
                nc.tensor.matmul(
                    av[:, off:512], vs[:, g1 * 65:(g1 + 1) * 65],
                    pt[:, 512:512 + w],
                    start=False, stop=(p == npairs - 1),
                )

            # software pipeline: emit S(p+1) ahead of AV(p)
            sts = {0: s_pair(0)}
            pts = {}
            for p in range(npairs):
                pts[p] = px_pair(p, sts.pop(p))
                if p + 1 < npairs:
                    sts[p + 1] = s_pair(p + 1)
                av_pair(p, pts.pop(p))

            oc = ocp.tile([65, 512], F32)
            nc.vector.tensor_copy(oc[:], av[:])
            nc.scalar.dma_start(o[c, :, :], oc[:])

        passA_chunk(0)
        passB_chunk(0)
        passA_chunk(1)
        passB_chunk(1)
        attn_seg(0)
        passA_chunk(2)
        passB_chunk(2)
        passA_chunk(3)
        passB_chunk(3)
        attn_seg(1)
        passA_chunk(4)
        passB_chunk(4)
        passA_chunk(5)
        passB_chunk(5)
        attn_seg(2)
        passA_chunk(6)
        passB_chunk(6)
        passA_chunk(7)
        passB_chunk(7)
        attn_seg(3)

    nc.compile()
    return nc


def _get_nc():
    global _CACHED_NC
    if _CACHED_NC is None:
        _CACHED_NC = build_nc()
    return _CACHED_NC


def _host_inputs(x, wq, bq, wk, bk, wv, bv):
    bf = ml_dtypes.bfloat16
    wqq = np.concatenate([wq, wq], axis=1)      # [768, 128]
    wkv = np.concatenate([wv, wk], axis=1)
    # interleave to [in-chunk partition, (chunk, out_col)]
    wqq = np.ascontiguousarray(
        wqq.reshape(NCC, 128, 128).transpose(1, 0, 2).reshape(128, NCC * 128)
    ).astype(bf)
    wkv = np.ascontiguousarray(
        wkv.reshape(NCC, 128, 128).transpose(1, 0, 2).reshape(128, NCC * 128)
    ).astype(bf)
    bqq = np.concatenate([bq, bq])[:, None].astype(np.float32)
    maska = np.triu(np.ones((128, 128), np.float32)).astype(bf)
    idnb = np.eye(64, dtype=np.float32).astype(bf)
    xbf = np.ascontiguousarray(x).astype(bf)

    in_maps = []
    for core in range(8):
        b, j = core // 2, core % 2
        if j == 0:
            xdev = np.concatenate(
                [np.zeros((128, DIN), bf), xbf[b][: SEQ - 128]], axis=0
            )
            ps = np.zeros((128, 1), np.float32)
        else:
            xdev = xbf[b]
            ps = np.ones((128, 1), np.float32)
        in_maps.append({
            "x": np.ascontiguousarray(xdev),
            "wqq": wqq, "wkv": wkv, "bqq": bqq,
            "pads": ps, "maska": maska, "idnb": idnb,
        })
    return in_maps


def _assemble(results, bv):
    out = np.empty((4, SEQ, DOUT), np.float32)
    for core in range(8):
        b, j = core // 2, core % 2
        od = results[core]["o"]  # [NQC, 65, 512]
        for c in range(NQC):
            num = od[c, 0:64, :].astype(np.float64)
            den = od[c, 64, :].astype(np.float64)
            oc = (num / den + bv[:, None]).T.astype(np.float32)  # [512, 64]
            for t in range(4):
                r0 = (8 * c + 2 * t + j) * 128
                out[b, r0:r0 + 128] = oc[t * 128:(t + 1) * 128]
    return out


def kernel(x, wq, bq, wk, bk, wv, bv):
    x = np.asarray(x, dtype=np.float32)
    args = [np.asarray(a, dtype=np.float32) for a in (wq, bq, wk, bk, wv, bv)]
    nc = _get_nc()
    in_maps = _host_inputs(x, *args)
    br = run_bass_kernel_spmd(nc, in_maps, core_ids=list(range(8)))
    return _assemble(br.results, args[5].astype(np.float64))


# revision 14
# speedup vs baseline: 1.1599x; 1.1599x over previous
"""Trainium2 Bass kernel: causal attention (QKV projection + causal softmax + AV).

Problem: x[4, 4096, 768] fp32, per-head projections to d=64, full causal
attention per batch, output [4, 4096, 64] fp32.

Sharding: 8 cores = 4 batches x 2 parity groups. Core (b, j) computes the
output rows of batch b whose 128-row block index i satisfies i % 2 == j.
One uniform SPMD program: for j=0 cores the host shifts x down by one
128-row block (prepending zeros), which makes the causal structure of both
parities identical in device coordinates (device q-blocks are always the odd
blocks 1,3,...,31; k-slot g holds true block g-1 for j=0 and g for j=1; the
dead slot 0 of j=0 is zeroed post-exp with a per-core 0/1 scale).

Device pipeline per core (all matmuls bf16, fp32 PSUM accumulation):
  P1 (per 512-row seq chunk): one 3D-output DMA-transpose yields x^T for the
     chunk (SP HWDGE queue, issued first); two matmul passes with stationary
     [wq|wq] and [wv|wk] produce Q^T (own q-blocks, both partition halves),
     K^T (high half, HWDGE-duplicated even slots to the low half) and V^T
     (PE-transposed into V' = [V | 1]). Bias adds run on Pool.
  P2 (per 512-col q chunk): for consecutive k-slot pairs, two concurrent
     row-tiled matmuls K^T_g.T @ Q^T produce S^T; one merged exp on ACT
     (scale 1/8, AP [128, 2, w]); causal-diagonal 128x128 tri mask and the
     j=0 dead-slot kill on Pool; AV accumulates V'.T @ P^T into a [65, 512]
     PSUM tile whose row 64 is the softmax denominator. The inner loop is
     software-pipelined: S(p+1) is emitted before AV(p) so PE never waits
     on the exp. The unnormalized [65, 512] tiles go to DRAM; the host
     divides and transposes.
All non-transpose DMAs (weights, dups, output stores) ride the ACT HWDGE
queue, whose descriptor generation does not occupy the ACT execution unit.
"""

import numpy as np
import ml_dtypes
from contextlib import ExitStack

import concourse.bass as bass
import concourse.mybir as mybir
import concourse.tile as tile
from concourse import bacc
from concourse.bass_utils import run_bass_kernel_spmd

F32 = mybir.dt.float32
BF16 = mybir.dt.bfloat16

SEQ = 4096
DIN = 768
DOUT = 64
NCC = DIN // 128          # 6 contraction chunks
NSC = SEQ // 512          # 8 seq chunks (projection granularity)
NBLK = SEQ // 128         # 32 k-slots
NQC = 4                   # q chunks of 512 local columns (2048 own q rows)
SCALE = 1.0 / 8.0
EXPF = mybir.ActivationFunctionType.Exp

_CACHED_NC = None


def build_nc():
    nc = bacc.Bacc("TRN2", target_bir_lowering=False, debug=False)

    x = nc.dram_tensor("x", [SEQ, DIN], BF16, kind="ExternalInput")
    # host pre-interleaves weights to [in-chunk partition, (chunk, out_col)]
    wqq = nc.dram_tensor("wqq", [128, NCC * 64], BF16, kind="ExternalInput")
    wkv = nc.dram_tensor("wkv", [128, NCC * 128], BF16, kind="ExternalInput")
    bqq = nc.dram_tensor("bqq", [128, 1], F32, kind="ExternalInput")     # [bq;bq]
    pads = nc.dram_tensor("pads", [128, 1], F32, kind="ExternalInput")   # 1 / 0
    maska = nc.dram_tensor("maska", [128, 128], BF16, kind="ExternalInput")
    idnb = nc.dram_tensor("idnb", [64, 64], BF16, kind="ExternalInput")
    o = nc.dram_tensor("o", [NQC, 65, 512], F32, kind="ExternalOutput")

    with tile.TileContext(nc) as tc, ExitStack() as ctx:
        cpool = ctx.enter_context(tc.tile_pool(name="const", bufs=1))
        vtp = ctx.enter_context(tc.tile_pool(name="vt", bufs=2))
        ptp = ctx.enter_context(tc.tile_pool(name="pt", bufs=3))
        ocp = ctx.enter_context(tc.tile_pool(name="oc", bufs=2))
        psproj = ctx.enter_context(tc.tile_pool(name="psproj", bufs=2, space="PSUM"))
        psst = ctx.enter_context(tc.tile_pool(name="psst", bufs=2, space="PSUM"))
        psav = ctx.enter_context(tc.tile_pool(name="psav", bufs=2, space="PSUM"))

        wqq_sb = cpool.tile([128, NCC * 64], BF16)
        wkv_sb = cpool.tile([128, NCC * 128], BF16)
        bqq_sb = cpool.tile([128, 1], F32)
        pads_sb = cpool.tile([128, 1], F32)
        mask_sb = cpool.tile([128, 128], BF16)
        idn_sb = cpool.tile([64, 64], BF16)
        kt2 = cpool.tile([128, NBLK * 128], BF16)   # rows 64-127: K^T; 0-63: V^T
        xtf = cpool.tile([128, NSC * NCC * 512], BF16)  # x^T, whole sequence
        qt = cpool.tile([128, 16 * 128], BF16)      # rows 64-127: Q^T own blocks
        vs = cpool.tile([128, NBLK * 65], BF16)     # V' = [V | 1] per k-slot

        # weights first on the ACT HWDGE queue so they win the DMA device
        # ahead of the first x^T transpose
        nc.scalar.dma_start(wqq_sb[:], wqq[:, :])
        nc.scalar.dma_start(wkv_sb[:], wkv[:, :])
        nc.scalar.dma_start(bqq_sb[:], bqq[:, :])
        nc.scalar.dma_start(pads_sb[:], pads[:, :])
        nc.scalar.dma_start(mask_sb[:], maska[:, :])
        nc.scalar.dma_start(idn_sb[:], idnb[:, :])
        # x^T transposes alone on the SP HWDGE queue
        for sc in range(NSC):
            nc.sync.dma_start_transpose(
                xtf[:, sc * NCC * 512:(sc + 1) * NCC * 512]
                .rearrange("p (cc s) -> p cc s", cc=NCC),
                x[sc * 512:(sc + 1) * 512, :],
            )
        # ones column of V'
        nc.vector.memset(
            vs[:].rearrange("p (g e) -> p g e", g=NBLK)[:, :, 64:65], 1.0
        )

        def xts(sc, cc):
            base = sc * NCC * 512 + cc * 512
            return xtf[:, base:base + 512]

        def passA_chunk(sc):
            """Q^T for own (odd) q-blocks of this chunk, wq stationary.

            Output lands on partitions 64-127 so the S matmuls can read both
            K^T and Q^T at PE tile row 64 (no low-half duplication needed).
            """
            qp = psproj.tile([128, 256], F32, tag="proj")
            for cc in range(NCC):
                rhs = (
                    xts(sc, cc)
                    .rearrange("p (a b s) -> p a b s", a=2, b=2)[:, :, 1, :]
                )
                nc.tensor.matmul(
                    qp[64:128, :], wqq_sb[:, cc * 64:(cc + 1) * 64], rhs,
                    start=(cc == 0), stop=(cc == NCC - 1),
                )
            nc.vector.tensor_scalar_add(
                qt[64:128, sc * 256:(sc + 1) * 256], qp[64:128, :], bqq_sb[64:128, :]
            )

        def passB_chunk(sc):
            """K^T (rows 64-127) and V^T (rows 0-63), [wv|wk] stationary.

            The k/v biases are dropped: bk adds a per-query constant to the
            scores (softmax-invariant) and bv a constant to the output (the
            host adds it after normalizing). One bias-free copy evacuates
            the whole PSUM tile into kt2; V sits in the low half just long
            enough for the V' transposes, then the dup DMA overwrites the
            even k-slots of the low half with K for the S matmuls.
            """
            kp = psproj.tile([128, 512], F32, tag="proj")
            for cc in range(NCC):
                nc.tensor.matmul(
                    kp[:], wkv_sb[:, cc * 128:(cc + 1) * 128],
                    xts(sc, cc),
                    start=(cc == 0), stop=(cc == NCC - 1),
                )
            nc.vector.tensor_copy(
                kt2[:, sc * 512:(sc + 1) * 512], kp[:]
            )
            # V' blocks via PE transpose (DMA-transpose is only HW-exact for
            # the whole-row DRAM-sourced x case)
            vp = psproj.tile([128, 256], BF16, tag="proj")
            for t in range(4):
                nc.tensor.transpose(
                    vp[:, t * 64:(t + 1) * 64],
                    kt2[0:64, sc * 512 + t * 128: sc * 512 + (t + 1) * 128],
                    idn_sb[:],
                )
            nc.vector.tensor_copy(
                vs[:].rearrange("p (g e) -> p g e", g=NBLK)[
                    :, sc * 4:(sc + 1) * 4, 0:64
                ],
                vp[:].rearrange("p (g e) -> p g e", g=4),
            )

        def attn_seg(c):
            """Attention for local q cols [c*512, (c+1)*512), k-slot pairs 0..4c+3."""
            npairs = 4 * c + 4           # k-slots 0..8c+7 in consecutive pairs
            av = psav.tile([65, 512], F32, tag="av")

            def geom(p):
                # both slots of pair p share width w, regions left-aligned
                k = p - 4 * c
                return 512 - 128 * k if k > 0 else 512

            def s_pair(p):
                w = geom(p)
                g0, g1 = 2 * p, 2 * p + 1
                st = psst.tile([128, 1024], F32, tag="st")
                nc.tensor.matmul(
                    st[:, 0:w], kt2[64:128, g0 * 128:(g0 + 1) * 128],
                    qt[64:128, c * 512 + 512 - w: (c + 1) * 512],
                    start=True, stop=True, tile_position=(64, 0),
                )
                nc.tensor.matmul(
                    st[:, 512:512 + w], kt2[64:128, g1 * 128:(g1 + 1) * 128],
                    qt[64:128, c * 512 + 512 - w: (c + 1) * 512],
                    start=True, stop=True, tile_position=(64, 0),
                )
                return st

            def px_pair(p, st):
                w = geom(p)
                pt = ptp.tile([128, 1024], BF16)
                nc.scalar.activation(
                    pt[:].rearrange("p (i n) -> p i n", i=2)[:, :, 0:w],
                    st[:].rearrange("p (i n) -> p i n", i=2)[:, :, 0:w],
                    EXPF, bias=0.0, scale=SCALE,
                )
                if p == 0:
                    # kill the j=0 dead slot 0 (pads = 0 there, 1 for j=1)
                    nc.gpsimd.tensor_scalar_mul(
                        pt[:, 0:512], pt[:, 0:512], pads_sb[:]
                    )
                if p >= npairs - 4:
                    # odd member of the last four pairs is causal-diagonal:
                    # its first 128 written cols are the triangular block
                    nc.gpsimd.tensor_mul(
                        pt[:, 512:640], pt[:, 512:640], mask_sb[:]
                    )
                return pt

            def av_pair(p, pt):
                w = geom(p)
                off = 512 - w
                g0, g1 = 2 * p, 2 * p + 1
                nc.tensor.matmul(
                    av[:, off:512], vs[:, g0 * 65:(g0 + 1) * 65],
                    pt[:, 0:w],
                    start=(p == 0), stop=False,
                )
                nc.tensor.matmul(
                    av[:, off:512], vs[:, g1 * 65:(g1 + 1) * 65],
                    pt[:, 512:512 + w],
                    start=False, stop=(p == npairs - 1),
                )

            # software pipeline: emit S(p+1) ahead of AV(p)
            sts = {0: s_pair(0)}
            pts = {}
            for p in range(npairs):
                pts[p] = px_pair(p, sts.pop(p))
                if p + 1 < npairs:
                    sts[p + 1] = s_pair(p + 1)
                av_pair(p, pts.pop(p))

            oc = ocp.tile([65, 512], F32)
            nc.vector.tensor_copy(oc[:], av[:])
            nc.scalar.dma_start(o[c, :, :], oc[:])

        passA_chunk(0)
        passB_chunk(0)
        passA_chunk(1)
        passB_chunk(1)
        attn_seg(0)
        passA_chunk(2)
        passB_chunk(2)
        passA_chunk(3)
        passB_chunk(3)
        attn_seg(1)
        passA_chunk(4)
        passB_chunk(4)
        passA_chunk(5)
        passB_chunk(5)
        attn_seg(2)
        passA_chunk(6)
        passB_chunk(6)
        passA_chunk(7)
        passB_chunk(7)
        attn_seg(3)

    nc.compile()
    return nc


def _get_nc():
    global _CACHED_NC
    if _CACHED_NC is None:
        _CACHED_NC = build_nc()
    return _CACHED_NC


def _host_inputs(x, wq, bq, wk, bk, wv, bv):
    bf = ml_dtypes.bfloat16
    wkv = np.concatenate([wv, wk], axis=1)
    # interleave to [in-chunk partition, (chunk, out_col)]
    wqq = np.ascontiguousarray(
        np.asarray(wq).reshape(NCC, 128, 64).transpose(1, 0, 2).reshape(128, NCC * 64)
    ).astype(bf)
    wkv = np.ascontiguousarray(
        wkv.reshape(NCC, 128, 128).transpose(1, 0, 2).reshape(128, NCC * 128)
    ).astype(bf)
    bqq = np.concatenate([bq, bq])[:, None].astype(np.float32)
    maska = np.triu(np.ones((128, 128), np.float32)).astype(bf)
    idnb = np.eye(64, dtype=np.float32).astype(bf)
    xbf = np.ascontiguousarray(x).astype(bf)

    in_maps = []
    for core in range(8):
        b, j = core // 2, core % 2
        if j == 0:
            xdev = np.concatenate(
                [np.zeros((128, DIN), bf), xbf[b][: SEQ - 128]], axis=0
            )
            ps = np.zeros((128, 1), np.float32)
        else:
            xdev = xbf[b]
            ps = np.ones((128, 1), np.float32)
        in_maps.append({
            "x": np.ascontiguousarray(xdev),
            "wqq": wqq, "wkv": wkv, "bqq": bqq,
            "pads": ps, "maska": maska, "idnb": idnb,
        })
    return in_maps


def _assemble(results, bv):
    out = np.empty((4, SEQ, DOUT), np.float32)
    for core in range(8):
        b, j = core // 2, core % 2
        od = results[core]["o"]  # [NQC, 65, 512]
        for c in range(NQC):
            num = od[c, 0:64, :].astype(np.float64)
            den = od[c, 64, :].astype(np.float64)
            oc = (num / den + bv[:, None]).T.astype(np.float32)  # [512, 64]
            for t in range(4):
                r0 = (8 * c + 2 * t + j) * 128
                out[b, r0:r0 + 128] = oc[t * 128:(t + 1) * 128]
    return out


def kernel(x, wq, bq, wk, bk, wv, bv):
    x = np.asarray(x, dtype=np.float32)
    args = [np.asarray(a, dtype=np.float32) for a in (wq, bq, wk, bk, wv, bv)]
    nc = _get_nc()
    in_maps = _host_inputs(x, *args)
    br = run_bass_kernel_spmd(nc, in_maps, core_ids=list(range(8)))
    return _assemble(br.results, args[5].astype(np.float64))


# revision 16
# speedup vs baseline: 1.3048x; 1.1248x over previous
"""Trainium2 Bass kernel: causal attention (QKV projection + causal softmax + AV).

Problem: x[4, 4096, 768] fp32, per-head projections to d=64, full causal
attention per batch, output [4, 4096, 64] fp32.

Sharding: 8 cores = 4 batches x 2 parity groups. Core (b, j) computes the
output rows of batch b whose 128-row block index i satisfies i % 2 == j.
One uniform SPMD program: for j=0 cores the host shifts x down by one
128-row block (prepending zeros), which makes the causal structure of both
parities identical in device coordinates (device q-blocks are always the odd
blocks 1,3,...,31; k-slot g holds true block g-1 for j=0 and g for j=1; the
dead slot 0 of j=0 is zeroed post-exp with a per-core 0/1 scale).

Device pipeline per core (all matmuls bf16, fp32 PSUM accumulation):
  P1 (per 512-row seq chunk): one 3D-output DMA-transpose yields x^T for the
     chunk (SP HWDGE queue, issued first); two matmul passes with stationary
     [wq|wq] and [wv|wk] produce Q^T (own q-blocks, both partition halves),
     K^T (high half, HWDGE-duplicated even slots to the low half) and V^T
     (PE-transposed into V' = [V | 1]). Bias adds run on Pool.
  P2 (per 512-col q chunk): for consecutive k-slot pairs, two concurrent
     row-tiled matmuls K^T_g.T @ Q^T produce S^T; one merged exp on ACT
     (scale 1/8, AP [128, 2, w]); causal-diagonal 128x128 tri mask and the
     j=0 dead-slot kill on Pool; AV accumulates V'.T @ P^T into a [65, 512]
     PSUM tile whose row 64 is the softmax denominator. The inner loop is
     software-pipelined: S(p+1) is emitted before AV(p) so PE never waits
     on the exp. The unnormalized [65, 512] tiles go to DRAM; the host
     divides and transposes.
All non-transpose DMAs (weights, dups, output stores) ride the ACT HWDGE
queue, whose descriptor generation does not occupy the ACT execution unit.
"""

import numpy as np
import ml_dtypes
from contextlib import ExitStack

import concourse.bass as bass
import concourse.mybir as mybir
import concourse.tile as tile
from concourse import bacc
from concourse.bass_utils import run_bass_kernel_spmd

F32 = mybir.dt.float32
BF16 = mybir.dt.bfloat16

SEQ = 4096
DIN = 768
DOUT = 64
NCC = DIN // 128          # 6 contraction chunks
NSC = SEQ // 512          # 8 seq chunks (projection granularity)
NBLK = SEQ // 128         # 32 k-slots
NQC = 4                   # q chunks of 512 local columns (2048 own q rows)
SCALE = 1.0 / 8.0
EXPF = mybir.ActivationFunctionType.Exp

_CACHED_NC = None


def build_nc():
    nc = bacc.Bacc("TRN2", target_bir_lowering=False, debug=False)

    x = nc.dram_tensor("x", [SEQ, DIN], BF16, kind="ExternalInput")
    # host pre-interleaves weights to [in-chunk partition, (chunk, out_col)]
    wqq = nc.dram_tensor("wqq", [128, NCC * 64], BF16, kind="ExternalInput")
    wkv = nc.dram_tensor("wkv", [128, NCC * 128], BF16, kind="ExternalInput")
    bqq = nc.dram_tensor("bqq", [128, 1], F32, kind="ExternalInput")     # [bq;bq]
    pads = nc.dram_tensor("pads", [128, 1], F32, kind="ExternalInput")   # 1 / 0
    maska = nc.dram_tensor("maska", [128, 128], BF16, kind="ExternalInput")
    idnb = nc.dram_tensor("idnb", [64, 64], BF16, kind="ExternalInput")
    o = nc.dram_tensor("o", [NQC, 65, 512], F32, kind="ExternalOutput")

    with tile.TileContext(nc) as tc, ExitStack() as ctx:
        cpool = ctx.enter_context(tc.tile_pool(name="const", bufs=1))
        vtp = ctx.enter_context(tc.tile_pool(name="vt", bufs=2))
        ptp = ctx.enter_context(tc.tile_pool(name="pt", bufs=3))
        ocp = ctx.enter_context(tc.tile_pool(name="oc", bufs=2))
        psproj = ctx.enter_context(tc.tile_pool(name="psproj", bufs=2, space="PSUM"))
        psst = ctx.enter_context(tc.tile_pool(name="psst", bufs=2, space="PSUM"))
        psav = ctx.enter_context(tc.tile_pool(name="psav", bufs=2, space="PSUM"))

        wqq_sb = cpool.tile([128, NCC * 64], BF16)
        wkv_sb = cpool.tile([128, NCC * 128], BF16)
        bqq_sb = cpool.tile([128, 1], F32)
        pads_sb = cpool.tile([128, 1], F32)
        mask_sb = cpool.tile([128, 128], BF16)
        idn_sb = cpool.tile([64, 64], BF16)
        kt2 = cpool.tile([128, NBLK * 128], BF16)   # rows 64-127: K^T; 0-63: V^T
        xtf = cpool.tile([128, NSC * NCC * 512], BF16)  # x^T, whole sequence
        qt = cpool.tile([128, 16 * 128], BF16)      # rows 64-127: Q^T own blocks
        vs = cpool.tile([128, NBLK * 65], BF16)     # V' = [V | 1] per k-slot

        # Weights then x^T transposes on the SP HWDGE queue: the in-order
        # queue makes the DMA-device order deterministic (weights land before
        # chunk 0, chunk c at ~4.5 + 2.7c us). Output stores go on this queue
        # too — their dependency waits must never hold the ACT sequencer,
        # which the exps need continuously.
        nc.sync.dma_start(wqq_sb[:], wqq[:, :])
        nc.sync.dma_start(wkv_sb[:], wkv[:, :])
        for sc in range(NSC):
            nc.sync.dma_start_transpose(
                xtf[:, sc * NCC * 512:(sc + 1) * NCC * 512]
                .rearrange("p (cc s) -> p cc s", cc=NCC),
                x[sc * 512:(sc + 1) * 512, :],
            )
        # non-critical constants race on the ACT HWDGE queue
        nc.scalar.dma_start(bqq_sb[:], bqq[:, :])
        nc.scalar.dma_start(pads_sb[:], pads[:, :])
        nc.scalar.dma_start(mask_sb[:], maska[:, :])
        nc.scalar.dma_start(idn_sb[:], idnb[:, :])
        # ones column of V'
        nc.vector.memset(
            vs[:].rearrange("p (g e) -> p g e", g=NBLK)[:, :, 64:65], 1.0
        )

        def xts(sc, cc):
            base = sc * NCC * 512 + cc * 512
            return xtf[:, base:base + 512]

        def passA_chunk(sc):
            """Q^T for own (odd) q-blocks of this chunk, wq stationary.

            Output lands on partitions 64-127 so the S matmuls can read both
            K^T and Q^T at PE tile row 64 (no low-half duplication needed).
            """
            qp = psproj.tile([128, 256], F32, tag="proj")
            for cc in range(NCC):
                rhs = (
                    xts(sc, cc)
                    .rearrange("p (a b s) -> p a b s", a=2, b=2)[:, :, 1, :]
                )
                nc.tensor.matmul(
                    qp[64:128, :], wqq_sb[:, cc * 64:(cc + 1) * 64], rhs,
                    start=(cc == 0), stop=(cc == NCC - 1),
                )
            nc.vector.tensor_scalar_add(
                qt[64:128, sc * 256:(sc + 1) * 256], qp[64:128, :], bqq_sb[64:128, :]
            )

        def passB_chunk(sc):
            """K^T (rows 64-127) and V^T (rows 0-63), [wv|wk] stationary.

            The k/v biases are dropped: bk adds a per-query constant to the
            scores (softmax-invariant) and bv a constant to the output (the
            host adds it after normalizing). One bias-free copy evacuates
            the whole PSUM tile into kt2; V sits in the low half just long
            enough for the V' transposes, then the dup DMA overwrites the
            even k-slots of the low half with K for the S matmuls.
            """
            kp = psproj.tile([128, 512], F32, tag="proj")
            for cc in range(NCC):
                nc.tensor.matmul(
                    kp[:], wkv_sb[:, cc * 128:(cc + 1) * 128],
                    xts(sc, cc),
                    start=(cc == 0), stop=(cc == NCC - 1),
                )
            nc.vector.tensor_copy(
                kt2[:, sc * 512:(sc + 1) * 512], kp[:]
            )
            # V' blocks via PE transpose (DMA-transpose is only HW-exact for
            # the whole-row DRAM-sourced x case)
            vp = psproj.tile([128, 256], BF16, tag="proj")
            for t in range(4):
                nc.tensor.transpose(
                    vp[:, t * 64:(t + 1) * 64],
                    kt2[0:64, sc * 512 + t * 128: sc * 512 + (t + 1) * 128],
                    idn_sb[:],
                )
            nc.vector.tensor_copy(
                vs[:].rearrange("p (g e) -> p g e", g=NBLK)[
                    :, sc * 4:(sc + 1) * 4, 0:64
                ],
                vp[:].rearrange("p (g e) -> p g e", g=4),
            )

        def attn_seg(c):
            """Attention for local q cols [c*512, (c+1)*512), k-slot pairs 0..4c+3."""
            npairs = 4 * c + 4           # k-slots 0..8c+7 in consecutive pairs
            av = psav.tile([65, 512], F32, tag="av")

            def geom(p):
                # both slots of pair p share width w, regions left-aligned
                k = p - 4 * c
                return 512 - 128 * k if k > 0 else 512

            def s_pair(p):
                w = geom(p)
                g0, g1 = 2 * p, 2 * p + 1
                st = psst.tile([128, 1024], F32, tag="st")
                nc.tensor.matmul(
                    st[:, 0:w], kt2[64:128, g0 * 128:(g0 + 1) * 128],
                    qt[64:128, c * 512 + 512 - w: (c + 1) * 512],
                    start=True, stop=True, tile_position=(64, 0),
                )
                nc.tensor.matmul(
                    st[:, 512:512 + w], kt2[64:128, g1 * 128:(g1 + 1) * 128],
                    qt[64:128, c * 512 + 512 - w: (c + 1) * 512],
                    start=True, stop=True, tile_position=(64, 0),
                )
                return st

            def px_pair(p, st):
                w = geom(p)
                pt = ptp.tile([128, 1024], BF16)
                nc.scalar.activation(
                    pt[:].rearrange("p (i n) -> p i n", i=2)[:, :, 0:w],
                    st[:].rearrange("p (i n) -> p i n", i=2)[:, :, 0:w],
                    EXPF, bias=0.0, scale=SCALE,
                )
                if p == 0:
                    # kill the j=0 dead slot 0 (pads = 0 there, 1 for j=1)
                    nc.gpsimd.tensor_scalar_mul(
                        pt[:, 0:512], pt[:, 0:512], pads_sb[:]
                    )
                if p >= npairs - 4:
                    # odd member of the last four pairs is causal-diagonal:
                    # its first 128 written cols are the triangular block
                    nc.gpsimd.tensor_mul(
                        pt[:, 512:640], pt[:, 512:640], mask_sb[:]
                    )
                return pt

            def av_pair(p, pt):
                w = geom(p)
                off = 512 - w
                g0, g1 = 2 * p, 2 * p + 1
                nc.tensor.matmul(
                    av[:, off:512], vs[:, g0 * 65:(g0 + 1) * 65],
                    pt[:, 0:w],
                    start=(p == 0), stop=False,
                )
                nc.tensor.matmul(
                    av[:, off:512], vs[:, g1 * 65:(g1 + 1) * 65],
                    pt[:, 512:512 + w],
                    start=False, stop=(p == npairs - 1),
                )

            # software pipeline: emit S(p+1) ahead of AV(p)
            sts = {0: s_pair(0)}
            pts = {}
            for p in range(npairs):
                pts[p] = px_pair(p, sts.pop(p))
                if p + 1 < npairs:
                    sts[p + 1] = s_pair(p + 1)
                av_pair(p, pts.pop(p))

            oc = ocp.tile([65, 512], F32)
            nc.vector.tensor_copy(oc[:], av[:])
            nc.sync.dma_start(o[c, :, :], oc[:])

        passA_chunk(0)
        passB_chunk(0)
        passA_chunk(1)
        passB_chunk(1)
        attn_seg(0)
        passA_chunk(2)
        passB_chunk(2)
        passA_chunk(3)
        passB_chunk(3)
        attn_seg(1)
        passA_chunk(4)
        passB_chunk(4)
        passA_chunk(5)
        passB_chunk(5)
        attn_seg(2)
        passA_chunk(6)
        passB_chunk(6)
        passA_chunk(7)
        passB_chunk(7)
        attn_seg(3)

    nc.compile()
    return nc


def _get_nc():
    global _CACHED_NC
    if _CACHED_NC is None:
        _CACHED_NC = build_nc()
    return _CACHED_NC


def _host_inputs(x, wq, bq, wk, bk, wv, bv):
    bf = ml_dtypes.bfloat16
    wkv = np.concatenate([wv, wk], axis=1)
    # interleave to [in-chunk partition, (chunk, out_col)]
    wqq = np.ascontiguousarray(
        np.asarray(wq).reshape(NCC, 128, 64).transpose(1, 0, 2).reshape(128, NCC * 64)
    ).astype(bf)
    wkv = np.ascontiguousarray(
        wkv.reshape(NCC, 128, 128).transpose(1, 0, 2).reshape(128, NCC * 128)
    ).astype(bf)
    bqq = np.concatenate([bq, bq])[:, None].astype(np.float32)
    maska = np.triu(np.ones((128, 128), np.float32)).astype(bf)
    idnb = np.eye(64, dtype=np.float32).astype(bf)
    xbf = np.ascontiguousarray(x).astype(bf)

    in_maps = []
    for core in range(8):
        b, j = core // 2, core % 2
        if j == 0:
            xdev = np.concatenate(
                [np.zeros((128, DIN), bf), xbf[b][: SEQ - 128]], axis=0
            )
            ps = np.zeros((128, 1), np.float32)
        else:
            xdev = xbf[b]
            ps = np.ones((128, 1), np.float32)
        in_maps.append({
            "x": np.ascontiguousarray(xdev),
            "wqq": wqq, "wkv": wkv, "bqq": bqq,
            "pads": ps, "maska": maska, "idnb": idnb,
        })
    return in_maps


def _assemble(results, bv):
    out = np.empty((4, SEQ, DOUT), np.float32)
    for core in range(8):
        b, j = core // 2, core % 2
        od = results[core]["o"]  # [NQC, 65, 512]
        for c in range(NQC):
            num = od[c, 0:64, :].astype(np.float64)
            den = od[c, 64, :].astype(np.float64)
            oc = (num / den + bv[:, None]).T.astype(np.float32)  # [512, 64]
            for t in range(4):
                r0 = (8 * c + 2 * t + j) * 128
                out[b, r0:r0 + 128] = oc[t * 128:(t + 1) * 128]
    return out


def kernel(x, wq, bq, wk, bk, wv, bv):
    x = np.asarray(x, dtype=np.float32)
    args = [np.asarray(a, dtype=np.float32) for a in (wq, bq, wk, bk, wv, bv)]
    nc = _get_nc()
    in_maps = _host_inputs(x, *args)
    br = run_bass_kernel_spmd(nc, in_maps, core_ids=list(range(8)))
    return _assemble(br.results, args[5].astype(np.float64))


# revision 22
# speedup vs baseline: 1.3110x; 1.0048x over previous
"""Trainium2 Bass kernel: causal attention (QKV projection + causal softmax + AV).

Problem: x[4, 4096, 768] fp32, per-head projections to d=64, full causal
attention per batch, output [4, 4096, 64] fp32.

Sharding: 8 cores = 4 batches x 2 parity groups. Core (b, j) computes the
output rows of batch b whose 128-row block index i satisfies i % 2 == j.
One uniform SPMD program: for j=0 cores the host shifts x down by one
128-row block (prepending zeros), which makes the causal structure of both
parities identical in device coordinates (device q-blocks are always the odd
blocks 1,3,...,31; k-slot g holds true block g-1 for j=0 and g for j=1; the
dead slot 0 of j=0 is zeroed post-exp with a per-core 0/1 scale).

Device pipeline per core (all matmuls bf16, fp32 PSUM accumulation):
  P1 (per 512-row seq chunk): one 3D-output DMA-transpose yields x^T for the
     chunk (SP HWDGE queue, issued first); two matmul passes with stationary
     [wq|wq] and [wv|wk] produce Q^T (own q-blocks, both partition halves),
     K^T (high half, HWDGE-duplicated even slots to the low half) and V^T
     (PE-transposed into V' = [V | 1]). Bias adds run on Pool.
  P2 (per 512-col q chunk): for consecutive k-slot pairs, two concurrent
     row-tiled matmuls K^T_g.T @ Q^T produce S^T; one merged exp on ACT
     (scale 1/8, AP [128, 2, w]); causal-diagonal 128x128 tri mask and the
     j=0 dead-slot kill on Pool; AV accumulates V'.T @ P^T into a [65, 512]
     PSUM tile whose row 64 is the softmax denominator. The inner loop is
     software-pipelined: S(p+1) is emitted before AV(p) so PE never waits
     on the exp. The unnormalized [65, 512] tiles go to DRAM; the host
     divides and transposes.
All non-transpose DMAs (weights, dups, output stores) ride the ACT HWDGE
queue, whose descriptor generation does not occupy the ACT execution unit.
"""

import numpy as np
import ml_dtypes
from contextlib import ExitStack

import concourse.bass as bass
import concourse.mybir as mybir
import concourse.tile as tile
from concourse import bacc
from concourse.bass_utils import run_bass_kernel_spmd

F32 = mybir.dt.float32
BF16 = mybir.dt.bfloat16

SEQ = 4096
DIN = 768
DOUT = 64
NCC = DIN // 128          # 6 contraction chunks
NSC = SEQ // 512          # 8 seq chunks (projection granularity)
NBLK = SEQ // 128         # 32 k-slots
NQC = 4                   # q chunks of 512 local columns (2048 own q rows)
SCALE = 1.0 / 8.0
EXPF = mybir.ActivationFunctionType.Exp

_CACHED_NC = None


def build_nc():
    nc = bacc.Bacc("TRN2", target_bir_lowering=False, debug=False)

    x = nc.dram_tensor("x", [SEQ, DIN], BF16, kind="ExternalInput")
    # host pre-interleaves weights to [in-chunk partition, (chunk, out_col)]
    wqq = nc.dram_tensor("wqq", [128, NCC * 64], BF16, kind="ExternalInput")
    wkv = nc.dram_tensor("wkv", [128, NCC * 128], BF16, kind="ExternalInput")
    bqq = nc.dram_tensor("bqq", [128, 1], F32, kind="ExternalInput")     # [bq;bq]
    pads = nc.dram_tensor("pads", [128, 1], F32, kind="ExternalInput")   # 1 / 0
    maska = nc.dram_tensor("maska", [128, 128], BF16, kind="ExternalInput")
    idnb = nc.dram_tensor("idnb", [64, 64], BF16, kind="ExternalInput")
    o = nc.dram_tensor("o", [NQC, 65, 512], F32, kind="ExternalOutput")

    with tile.TileContext(nc) as tc, ExitStack() as ctx:
        cpool = ctx.enter_context(tc.tile_pool(name="const", bufs=1))
        vtp = ctx.enter_context(tc.tile_pool(name="vt", bufs=2))
        ptp = ctx.enter_context(tc.tile_pool(name="pt", bufs=3))
        ocp = ctx.enter_context(tc.tile_pool(name="oc", bufs=2))
        psproj = ctx.enter_context(tc.tile_pool(name="psproj", bufs=2, space="PSUM"))
        psst = ctx.enter_context(tc.tile_pool(name="psst", bufs=2, space="PSUM"))
        psav = ctx.enter_context(tc.tile_pool(name="psav", bufs=2, space="PSUM"))

        wqq_sb = cpool.tile([128, NCC * 64], BF16)
        wkv_sb = cpool.tile([128, NCC * 128], BF16)
        bqq_sb = cpool.tile([128, 1], F32)
        pads_sb = cpool.tile([128, 1], F32)
        mask_sb = cpool.tile([128, 128], BF16)
        idn_sb = cpool.tile([64, 64], BF16)
        kt2 = cpool.tile([128, NBLK * 128], BF16)   # rows 64-127: K^T; 0-63: V^T
        xtf = cpool.tile([128, NSC * NCC * 512], BF16)  # x^T, whole sequence
        qt = cpool.tile([128, 16 * 128], BF16)      # rows 64-127: Q^T own blocks
        vs = cpool.tile([128, NBLK * 65], BF16)     # V' = [V | 1] per k-slot

        # Weights then x^T transposes on the SP HWDGE queue: the in-order
        # queue makes the DMA-device order deterministic (weights land before
        # chunk 0). The first two chunks are transposed in thirds so the
        # projection accumulation chains can start ~1us after the weights.
        # Output stores go on this queue too — their dependency waits must
        # never hold the ACT sequencer, which the exps need continuously.
        nc.sync.dma_start(wqq_sb[:], wqq[:, :])
        nc.sync.dma_start(wkv_sb[:], wkv[:, :])
        for sc in range(NSC):
            pieces = 3 if sc < 2 else 1
            cpp = NCC // pieces
            for pc in range(pieces):
                nc.sync.dma_start_transpose(
                    xtf[:, (sc * NCC + pc * cpp) * 512:
                        (sc * NCC + (pc + 1) * cpp) * 512]
                    .rearrange("p (cc s) -> p cc s", cc=cpp),
                    x[sc * 512:(sc + 1) * 512,
                      pc * cpp * 128:(pc + 1) * cpp * 128],
                )
        # non-critical constants race on the ACT HWDGE queue
        nc.scalar.dma_start(bqq_sb[:], bqq[:, :])
        nc.scalar.dma_start(pads_sb[:], pads[:, :])
        nc.scalar.dma_start(mask_sb[:], maska[:, :])
        nc.scalar.dma_start(idn_sb[:], idnb[:, :])
        # ones column of V'
        nc.vector.memset(
            vs[:].rearrange("p (g e) -> p g e", g=NBLK)[:, :, 64:65], 1.0
        )

        def xts(sc, cc):
            base = sc * NCC * 512 + cc * 512
            return xtf[:, base:base + 512]

        def emit_passA(sc):
            """Q^T for own (odd) q-blocks of this chunk, wq stationary.

            Output lands on partitions 64-127 so the S matmuls can read both
            K^T and Q^T at PE tile row 64 (no low-half duplication needed).
            Returns thunks so the pass can interleave with attention pairs.
            """
            state = {}

            def mms(c0, c1):
                if c0 == 0:
                    state["qp"] = psproj.tile([128, 256], F32, tag="proj", name="qp")
                qp = state["qp"]
                for cc in range(c0, c1):
                    rhs = (
                        xts(sc, cc)
                        .rearrange("p (a b s) -> p a b s", a=2, b=2)[:, :, 1, :]
                    )
                    nc.tensor.matmul(
                        qp[64:128, :], wqq_sb[:, cc * 64:(cc + 1) * 64], rhs,
                        start=(cc == 0), stop=(cc == NCC - 1),
                    )
                if c1 == NCC:
                    nc.vector.tensor_scalar_add(
                        qt[64:128, sc * 256:(sc + 1) * 256],
                        qp[64:128, :], bqq_sb[64:128, :]
                    )

            return [lambda: mms(0, 3), lambda: mms(3, NCC)]

        def emit_passB(sc):
            """K^T (rows 64-127) and V^T (rows 0-63), [wv|wk] stationary.

            The k/v biases are dropped: bk adds a per-query constant to the
            scores (softmax-invariant) and bv a constant to the output (the
            host adds it after normalizing). One bias-free copy evacuates
            the whole PSUM tile into kt2; V parks in the low half, which the
            S matmuls never read.
            """
            state = {}

            def mms(c0, c1):
                if c0 == 0:
                    state["kp"] = psproj.tile([128, 512], F32, tag="proj", name="kp")
                kp = state["kp"]
                for cc in range(c0, c1):
                    nc.tensor.matmul(
                        kp[:], wkv_sb[:, cc * 128:(cc + 1) * 128],
                        xts(sc, cc),
                        start=(cc == 0), stop=(cc == NCC - 1),
                    )
                if c1 == NCC:
                    nc.vector.tensor_copy(
                        kt2[:, sc * 512:(sc + 1) * 512], kp[:]
                    )

            def vpass():
                # V' blocks via PE transpose (DMA-transpose is only HW-exact
                # for the whole-row DRAM-sourced x case)
                vp = psproj.tile([128, 256], BF16, tag="proj")
                for t in range(4):
                    nc.tensor.transpose(
                        vp[:, t * 64:(t + 1) * 64],
                        kt2[0:64, sc * 512 + t * 128: sc * 512 + (t + 1) * 128],
                        idn_sb[:],
                    )
                nc.vector.tensor_copy(
                    vs[:].rearrange("p (g e) -> p g e", g=NBLK)[
                        :, sc * 4:(sc + 1) * 4, 0:64
                    ],
                    vp[:].rearrange("p (g e) -> p g e", g=4),
                )

            return [lambda: mms(0, 3), lambda: mms(3, NCC), vpass]

        def attn_seg(c, fill=()):
            """Attention for local q cols [c*512, (c+1)*512), k-slot pairs 0..4c+3.

            `fill` thunks (projection work for later chunks) are spread across
            the pair loop, emitted between S(p+1) and AV(p) so the PE chews
            on them while the exp of pair p is still in flight on ACT.
            """
            npairs = 4 * c + 4           # k-slots 0..8c+7 in consecutive pairs
            av = psav.tile([65, 512], F32, tag="av")

            def geom(p):
                # both slots of pair p share width w, regions left-aligned
                k = p - 4 * c
                return 512 - 128 * k if k > 0 else 512

            def s_pair(p):
                w = geom(p)
                g0, g1 = 2 * p, 2 * p + 1
                st = psst.tile([128, 1024], F32, tag="st")
                nc.tensor.matmul(
                    st[:, 0:w], kt2[64:128, g0 * 128:(g0 + 1) * 128],
                    qt[64:128, c * 512 + 512 - w: (c + 1) * 512],
                    start=True, stop=True, tile_position=(64, 0),
                )
                nc.tensor.matmul(
                    st[:, 512:512 + w], kt2[64:128, g1 * 128:(g1 + 1) * 128],
                    qt[64:128, c * 512 + 512 - w: (c + 1) * 512],
                    start=True, stop=True, tile_position=(64, 0),
                )
                return st

            def px_pair(p, st):
                w = geom(p)
                pt = ptp.tile([128, 1024], BF16)
                nc.scalar.activation(
                    pt[:].rearrange("p (i n) -> p i n", i=2)[:, :, 0:w],
                    st[:].rearrange("p (i n) -> p i n", i=2)[:, :, 0:w],
                    EXPF, bias=0.0, scale=SCALE,
                )
                if p == 0:
                    # kill the j=0 dead slot 0 (pads = 0 there, 1 for j=1)
                    nc.gpsimd.tensor_scalar_mul(
                        pt[:, 0:512], pt[:, 0:512], pads_sb[:]
                    )
                if p >= npairs - 4:
                    # odd member of the last four pairs is causal-diagonal:
                    # its first 128 written cols are the triangular block
                    nc.gpsimd.tensor_mul(
                        pt[:, 512:640], pt[:, 512:640], mask_sb[:]
                    )
                return pt

            def av_pair(p, pt):
                w = geom(p)
                off = 512 - w
                g0, g1 = 2 * p, 2 * p + 1
                nc.tensor.matmul(
                    av[:, off:512], vs[:, g0 * 65:(g0 + 1) * 65],
                    pt[:, 0:w],
                    start=(p == 0), stop=False,
                )
                nc.tensor.matmul(
                    av[:, off:512], vs[:, g1 * 65:(g1 + 1) * 65],
                    pt[:, 512:512 + w],
                    start=False, stop=(p == npairs - 1),
                )

            # software pipeline: emit S(p+1) ahead of AV(p), filler in between
            fill = list(fill)
            fi = 0
            sts = {0: s_pair(0)}
            for p in range(npairs):
                pt = px_pair(p, sts.pop(p))
                if p + 1 < npairs:
                    sts[p + 1] = s_pair(p + 1)
                # front-loaded: all filler lands in the first ~half of the
                # pairs, safely ahead of the pairs whose k-slots it produces
                want = min(len(fill), (p + 1) * 2 * len(fill) // npairs)
                while fi < want:
                    fill[fi]()
                    fi += 1
                av_pair(p, pt)

            oc = ocp.tile([65, 512], F32)
            nc.vector.tensor_copy(oc[:], av[:])
            nc.sync.dma_start(o[c, :, :], oc[:])

        # chunks 0-3 projected up front (segment c needs q-chunks 2c, 2c+1
        # and k-chunks <= 2c+1); later chunks interleave into the attention
        # segments, with B6/B7 deferred into the ACT-bound final segment
        # (k-slots 24-31 are only touched from pair 12 on).
        for t in emit_passA(0) + emit_passB(0) + emit_passA(1) + emit_passB(1):
            t()
        attn_seg(0)
        for t in emit_passA(2) + emit_passB(2) + emit_passA(3) + emit_passB(3):
            t()
        attn_seg(1, emit_passA(4) + emit_passB(4) + emit_passA(5))
        attn_seg(2, emit_passB(5) + emit_passA(6) + emit_passA(7))
        attn_seg(3, emit_passB(6) + emit_passB(7))

    nc.compile()
    return nc


def _get_nc():
    global _CACHED_NC
    if _CACHED_NC is None:
        _CACHED_NC = build_nc()
    return _CACHED_NC


def _host_inputs(x, wq, bq, wk, bk, wv, bv):
    bf = ml_dtypes.bfloat16
    wkv = np.concatenate([wv, wk], axis=1)
    # interleave to [in-chunk partition, (chunk, out_col)]
    wqq = np.ascontiguousarray(
        np.asarray(wq).reshape(NCC, 128, 64).transpose(1, 0, 2).reshape(128, NCC * 64)
    ).astype(bf)
    wkv = np.ascontiguousarray(
        wkv.reshape(NCC, 128, 128).transpose(1, 0, 2).reshape(128, NCC * 128)
    ).astype(bf)
    bqq = np.concatenate([bq, bq])[:, None].astype(np.float32)
    maska = np.triu(np.ones((128, 128), np.float32)).astype(bf)
    idnb = np.eye(64, dtype=np.float32).astype(bf)
    xbf = np.ascontiguousarray(x).astype(bf)

    in_maps = []
    for core in range(8):
        b, j = core // 2, core % 2
        if j == 0:
            xdev = np.concatenate(
                [np.zeros((128, DIN), bf), xbf[b][: SEQ - 128]], axis=0
            )
            ps = np.zeros((128, 1), np.float32)
        else:
            xdev = xbf[b]
            ps = np.ones((128, 1), np.float32)
        in_maps.append({
            "x": np.ascontiguousarray(xdev),
            "wqq": wqq, "wkv": wkv, "bqq": bqq,
            "pads": ps, "maska": maska, "idnb": idnb,
        })
    return in_maps


def _assemble(results, bv):
    out = np.empty((4, SEQ, DOUT), np.float32)
    for core in range(8):
        b, j = core // 2, core % 2
        od = results[core]["o"]  # [NQC, 65, 512]
        for c in range(NQC):
            num = od[c, 0:64, :].astype(np.float64)
            den = od[c, 64, :].astype(np.float64)
            oc = (num / den + bv[:, None]).T.astype(np.float32)  # [512, 64]
            for t in range(4):
                r0 = (8 * c + 2 * t + j) * 128
                out[b, r0:r0 + 128] = oc[t * 128:(t + 1) * 128]
    return out


def kernel(x, wq, bq, wk, bk, wv, bv):
    x = np.asarray(x, dtype=np.float32)
    args = [np.asarray(a, dtype=np.float32) for a in (wq, bq, wk, bk, wv, bv)]
    nc = _get_nc()
    in_maps = _host_inputs(x, *args)
    br = run_bass_kernel_spmd(nc, in_maps, core_ids=list(range(8)))
    return _assemble(br.results, args[5].astype(np.float64))
